# revision 15
# baseline (speedup 1.0000x reference)
"""DelayGNNStage Trainium2 kernel: 3-layer, 2-hop message-passing GNN.

Strategy (graph/data parallel over 8 NeuronCores):
  - Nodes are partitioned across cores by destination (12800 padded rows each).
  - Edges are sharded by dst core, sorted by (dst superblock, dst row).
  - Per 512-row dst superblock: gather source rows ([128,1]-offset indirect
    DMAs, one per 128-edge tile), build one-hot segment matrices on-chip
    (is_equal vs an iota constant), and aggregate via TensorE matmuls into
    PSUM as aggT[d, dst_window].
  - aggT @ (softmax(alpha)-scaled W) for both hops accumulates in PSUM,
    then relu + residual on the SBUF-resident x slice.
  - Updated slices are AllGathered between layers so the next layer's
    hop-1 (and later hop-2) gathers can read the full table.
"""

import time

import numpy as np

import concourse.bass as bass
import concourse.mybir as mybir
import concourse.mybir as mb
from concourse.tile import TileContext

# problem constants (hardcoded per contract)
N, E, D, T, K, NU = 100000, 1600000, 128, 3, 2, 1
NCORES = 8
NPC = 12800          # padded nodes per core (25 superblocks x 512)
NSB = NPC // 512     # superblocks per core
NPAD = NCORES * NPC  # 102400
SEG_TILES = 8        # tiles gathered/S-built per segment


def _split_multiwaits(nc):
    """Walrus in this container only accepts one sem-wait per instruction;
    hoist extras onto same-engine NoOps immediately before."""
    for fn in nc.m.functions:
        for bb in fn.blocks:
            newinsts = []
            for ins in bb.instructions:
                si = ins.sync_info
                try:
                    waits = list(si.on_wait) if si is not None else []
                except Exception:
                    waits = []
                if len(waits) > 1:
                    keep = waits[-1]
                    for w in waits[:-1]:
                        nop = mb.InstNoOp(
                            name=nc.get_next_instruction_name(), ins=[], outs=[])
                        nop.engine = ins.engine
                        nop.sync_info = mb.SyncInfo(on_wait=[w], on_update=[])
                        newinsts.append(nop)
                    ins.sync_info = mb.SyncInfo(
                        on_wait=[keep], on_update=list(si.on_update))
                newinsts.append(ins)
            bb.instructions = newinsts


def _build_schedule(src_by, dst_by):
    """Common (cross-core) tile schedule for one hop.

    src_by/dst_by: per core, per superblock: arrays of (src_padded,
    dst_local_in_sb) sorted by dst_local.

    Returns:
      ntiles: [NSB] list of tile counts (common across cores)
      wins:   per sb, list of window bases (len ntiles[sb])
      idx:    [NCORES][128, total_tiles] int32 gather row ids (pad -> 0)
      rel:    [NCORES][128, total_tiles] f32 dst_rel in [0,128) or -1 pad
    """
    ntiles = []
    wins = []
    per_core_cols_idx = [[] for _ in range(NCORES)]
    per_core_cols_rel = [[] for _ in range(NCORES)]
    for s in range(NSB):
        ptr = [0] * NCORES
        srcs = [src_by[c][s] for c in range(NCORES)]
        dsts = [dst_by[c][s] for c in range(NCORES)]
        lens = [len(x) for x in srcs]
        sb_wins = []
        while True:
            rem = [lens[c] - ptr[c] for c in range(NCORES)]
            if max(rem) == 0:
                break
            # window base: min over cores of next dst_local
            w = min(int(dsts[c][ptr[c]]) for c in range(NCORES) if rem[c] > 0)
            w = min(w, 512 - 128)
            sb_wins.append(w)
            for c in range(NCORES):
                p0 = ptr[c]
                # fill up to 128 edges with dst_local < w + 128
                hi = min(p0 + 128, lens[c])
                d = dsts[c]
                p1 = p0
                while p1 < hi and d[p1] < w + 128:
                    p1 += 1
                cnt = p1 - p0
                coli = np.zeros(128, dtype=np.int32)
                colr = np.full(128, -1.0, dtype=np.float32)
                if cnt:
                    coli[:cnt] = srcs[c][p0:p1]
                    colr[:cnt] = d[p0:p1] - w
                per_core_cols_idx[c].append(coli)
                per_core_cols_rel[c].append(colr)
                ptr[c] = p1
        ntiles.append(len(sb_wins))
        wins.append(sb_wins)
    idx = [np.stack(per_core_cols_idx[c], axis=1) for c in range(NCORES)]
    rel = [np.stack(per_core_cols_rel[c], axis=1).astype(np.float32)
           for c in range(NCORES)]
    return ntiles, wins, idx, rel


def _preprocess(x, edge_index, edge_attr, W, b, alpha):
    """Host-side sharding/scheduling. Returns per-core input maps and the
    common schedule metadata."""
    x = np.asarray(x, dtype=np.float32)
    ei = np.asarray(edge_index)
    ea = np.asarray(edge_attr)
    W = np.asarray(W, dtype=np.float32)
    b = np.asarray(b, dtype=np.float32)
    alpha = np.asarray(alpha, dtype=np.float32)

    # softmax over k, fold into W; fold bias rows
    aexp = np.exp(alpha - alpha.max(axis=1, keepdims=True))
    a = aexp / aexp.sum(axis=1, keepdims=True)          # [T, K]
    Ws = W * a[:, :, None, None]                        # [T, K, D, D]
    bias_rows = (a[:, :, None] * b).sum(axis=1)         # [T, D]
    has_bias = bool(np.abs(bias_rows).max() > 0)

    # node padding: original v -> core v//12500, padded id c*NPC + (v - c*12500)
    orig_per_core = N // NCORES  # 12500
    v = np.arange(N)
    core_of = v // orig_per_core
    pad_id = core_of * NPC + (v - core_of * orig_per_core)

    x_pad = np.zeros((NPAD, D), dtype=np.float32)
    x_pad[pad_id] = x
    # partition-major packing: X_pm[c*128+p, g*128+d] = x_pad[c*12800+g*128+p, d]
    x_pm = (x_pad.reshape(NCORES, NPC // 128, 128, D)
            .transpose(0, 2, 1, 3).reshape(NCORES * 128, NPC // 128 * D))

    src_p = pad_id[ei[0]]
    dst_p = pad_id[ei[1]]

    hops = []
    for k in (1, 2):
        sel = ea == k
        s_k, d_k = src_p[sel], dst_p[sel]
        dcore = d_k // NPC
        dloc = d_k - dcore * NPC
        src_by = [[] for _ in range(NCORES)]
        dst_by = [[] for _ in range(NCORES)]
        for c in range(NCORES):
            m = dcore == c
            sc, dc = s_k[m], dloc[m]
            sb = dc // 512
            dl = dc - sb * 512
            order = np.lexsort((dl, sb))
            sc, sb, dl = sc[order], sb[order], dl[order]
            bounds = np.searchsorted(sb, np.arange(NSB + 1))
            for s in range(NSB):
                lo, hi = bounds[s], bounds[s + 1]
                src_by[c].append(sc[lo:hi].astype(np.int32))
                dst_by[c].append(dl[lo:hi].astype(np.int32))
        ntiles, wins, idx, rel = _build_schedule(src_by, dst_by)
        # remap padded node id -> partition-major table row (q*100+g where
        # q = c*128+p, node = c*12800 + g*128 + p)
        for c in range(NCORES):
            s_ = idx[c]
            sc_ = s_ // NPC
            rem = s_ - sc_ * NPC
            g_ = rem // 128
            p_ = rem - g_ * 128
            idx[c] = ((sc_ * 128 + p_) * (NPC // 128) + g_).astype(np.int32)
        hops.append(dict(ntiles=ntiles, wins=wins, idx=idx, rel=rel))

    return dict(x_pad=x_pad, x_pm=x_pm, pad_id=pad_id, Ws=Ws,
                bias_rows=bias_rows, has_bias=has_bias, hops=hops)


def _build_kernel(meta, split=True):
    """Emit the bass kernel for the common schedule in `meta`."""
    hops = meta["hops"]
    has_bias = meta["has_bias"]
    T1 = sum(hops[0]["ntiles"])   # total tiles hop 1
    T2 = sum(hops[1]["ntiles"])

    nc = bass.Bass(num_devices=NCORES, num_swdge_queues=4)
    f32 = mybir.dt.float32

    # partition-major table layout: row q=(c*128+p), col (g*128+d) holds
    # node c*12800+g*128+p feature d; gather view row = q*100+g
    X0 = nc.dram_tensor("X0", [NCORES * 128, NPC // 128 * D], f32,
                        kind="ExternalInput")
    x0s = nc.dram_tensor("x0s", [128, NPC // 128 * D], f32,
                         kind="ExternalInput")
    idx1 = nc.dram_tensor("idx1", [128, T1], mybir.dt.int32, kind="ExternalInput")
    idx2 = nc.dram_tensor("idx2", [128, T2], mybir.dt.int32, kind="ExternalInput")
    rel1 = nc.dram_tensor("rel1", [128, T1], f32, kind="ExternalInput")
    rel2 = nc.dram_tensor("rel2", [128, T2], f32, kind="ExternalInput")
    Wd = nc.dram_tensor("Wd", [T * K, D, D], f32, kind="ExternalInput")
    IOTA = nc.dram_tensor("IOTA", [128, 128], f32, kind="ExternalInput")
    BIASD = nc.dram_tensor("BIASD", [T, 128, D], f32, kind="ExternalInput")
    Y = nc.dram_tensor("Y", [128, NPC // 128 * D], f32, kind="ExternalOutput")

    cc_in = [nc.dram_tensor(f"cc_in{t}", [128, NPC // 128 * D], f32)
             for t in range(T - 1)]
    cc_out = [nc.dram_tensor(f"cc_out{t}", [NCORES * 128, NPC // 128 * D], f32,
              addr_space="Shared") for t in range(T - 1)]

    # gather tables per (t, k): k=1 reads xs[t], k=2 reads xs[t-1] (t=0,1 -> x0)
    tables = {(0, 1): X0, (0, 2): X0,
              (1, 1): cc_out[0], (1, 2): X0,
              (2, 1): cc_out[1], (2, 2): cc_out[0]}

    idx_d = {1: idx1, 2: idx2}
    rel_d = {1: rel1, 2: rel2}

    with TileContext(nc) as tc:
        with (
            tc.tile_pool(name="const", bufs=1) as cpool,
            tc.tile_pool(name="blob", bufs=1) as bpool,
            tc.tile_pool(name="xres", bufs=1) as xpool,
            tc.tile_pool(name="agg2res", bufs=1) as a2pool,
            tc.tile_pool(name="mbuf", bufs=3) as mpool,
            tc.tile_pool(name="sbuf_s", bufs=3) as spool,
            tc.tile_pool(name="agg1", bufs=2) as a1pool,
            tc.tile_pool(name="relu", bufs=3) as rpool,
            tc.tile_pool(name="psumA", bufs=2, space="PSUM") as ppoolA,
            tc.tile_pool(name="psumB", bufs=2, space="PSUM") as ppoolB,
            tc.tile_pool(name="psumO", bufs=2, space="PSUM") as ppoolO,
        ):
            iota_sb = cpool.tile([128, 128], f32, name="iota_sb")
            nc.sync.dma_start(out=iota_sb[:], in_=IOTA[:])
            zero_sb = cpool.tile([128, 512], f32, name="zero_sb")
            nc.vector.memset(zero_sb[:], 0.0)
            w_sb = cpool.tile([128, T * K * D], f32, name="w_sb")
            nc.sync.dma_start(
                out=w_sb[:].rearrange("p (g d) -> p g d", d=D),
                in_=Wd[:].rearrange("g p d -> p g d"))
            if has_bias:
                bias_sb = cpool.tile([128, T * D], f32, name="bias_sb")
                nc.sync.dma_start(
                    out=bias_sb[:].rearrange("p (t d) -> p t d", d=D),
                    in_=BIASD[:].rearrange("t p d -> p t d"))

            idx_sb = {}
            rel_sb = {}
            for k, Tk in ((1, T1), (2, T2)):
                it = bpool.tile([128, Tk], mybir.dt.int32, name=f"idx_sb{k}")
                nc.sync.dma_start(out=it[:], in_=idx_d[k][:])
                rt = bpool.tile([128, Tk], f32, name=f"rel_sb{k}")
                nc.sync.dma_start(out=rt[:], in_=rel_d[k][:])
                idx_sb[k] = it
                rel_sb[k] = rt

            x_sl = xpool.tile([128, NPC], f32, name="x_sl")  # [p, blk*128+d]
            nc.sync.dma_start(out=x_sl[:], in_=x0s[:])

            agg2 = a2pool.tile([128, NSB * 512], f32, name="agg2")

            # tile column offsets per (k, sb)
            tile_off = {}
            for ki, k in enumerate((1, 2)):
                off = 0
                for s in range(NSB):
                    tile_off[(k, s)] = off
                    off += hops[ki]["ntiles"][s]

            def hop_aggregate(t, k, s, ppool, ptag):
                """Gather + segment matmuls for (layer t, hop k, superblock s).
                Returns the PSUM aggT tile [128, 512]."""
                ki = k - 1
                nt = hops[ki]["ntiles"][s]
                wins = hops[ki]["wins"][s]
                base = tile_off[(k, s)]
                psum = ppool.tile([128, 512], f32, space="PSUM",
                                  name=f"ps{t}_{k}_{s}", tag=ptag)
                nc.scalar.copy(out=psum[:], in_=zero_sb[:])
                table = tables[(t, k)][:].rearrange("q (g d) -> (q g) d", d=D)
                for g0 in range(0, nt, SEG_TILES):
                    g1 = min(g0 + SEG_TILES, nt)
                    ntg = g1 - g0
                    m = mpool.tile([128, SEG_TILES * 128], f32,
                                   name=f"m{t}_{k}_{s}_{g0}", tag="m")
                    for j in range(g0, g1):
                        nc.gpsimd.indirect_dma_start(
                            out=m[:, (j - g0) * 128:(j - g0 + 1) * 128],
                            out_offset=None,
                            in_=table,
                            in_offset=bass.IndirectOffsetOnAxis(
                                ap=idx_sb[k][:, base + j:base + j + 1], axis=0),
                        )
                    sm = spool.tile([128, SEG_TILES * 128], f32,
                                    name=f"s{t}_{k}_{s}_{g0}", tag="s")
                    nc.vector.tensor_tensor(
                        out=sm[:, :ntg * 128].rearrange("p (j c) -> p j c", c=128),
                        in0=rel_sb[k][:, base + g0:base + g1]
                            .to_broadcast([128, ntg, 128]),
                        in1=iota_sb[:].rearrange("p (j c) -> p j c", j=1)
                            .to_broadcast([128, ntg, 128]),
                        op=mybir.AluOpType.is_equal,
                    )
                    for j in range(g0, g1):
                        w = wins[j]
                        jj = j - g0
                        nc.tensor.matmul(
                            out=psum[:, w:w + 128],
                            lhsT=m[:, jj * 128:(jj + 1) * 128],
                            rhs=sm[:, jj * 128:(jj + 1) * 128],
                            start=False, stop=(j == nt - 1),
                            skip_group_check=True,
                        )
                return psum

            for t in range(T):
                # phase A: hop 2 (older table) -> resident agg2
                for s in range(NSB):
                    psA = hop_aggregate(t, 2, s, ppoolA, "psA")
                    nc.scalar.copy(out=agg2[:, s * 512:(s + 1) * 512], in_=psA[:])
                # phase B: hop 1 (fresh table) + combine + residual + store
                for s in range(NSB):
                    psB = hop_aggregate(t, 1, s, ppoolB, "psB")
                    a1 = a1pool.tile([128, 512], f32, name=f"a1_{t}_{s}", tag="a1")
                    nc.scalar.copy(out=a1[:], in_=psB[:])
                    for g in range(4):
                        op = ppoolO.tile([128, 128], f32, space="PSUM",
                                         name=f"op{t}_{s}_{g}", tag="op")
                        nc.tensor.matmul(
                            out=op[:], lhsT=a1[:, g * 128:(g + 1) * 128],
                            rhs=w_sb[:, (t * K + 0) * D:(t * K + 1) * D],
                            start=True, stop=False, skip_group_check=True)
                        nc.tensor.matmul(
                            out=op[:],
                            lhsT=agg2[:, s * 512 + g * 128: s * 512 + (g + 1) * 128],
                            rhs=w_sb[:, (t * K + 1) * D:(t * K + 2) * D],
                            start=False, stop=True, skip_group_check=True)
                        rt = rpool.tile([128, 128], f32,
                                        name=f"rt{t}_{s}_{g}", tag="rt")
                        if has_bias:
                            nc.vector.tensor_tensor(
                                out=rt[:], in0=op[:],
                                in1=bias_sb[:, t * D:(t + 1) * D],
                                op=mybir.AluOpType.add)
                            nc.scalar.activation(
                                out=rt[:], in_=rt[:],
                                func=mybir.ActivationFunctionType.Relu)
                        else:
                            nc.scalar.activation(
                                out=rt[:], in_=op[:],
                                func=mybir.ActivationFunctionType.Relu)
                        col = s * 512 + g * 128
                        nc.vector.tensor_add(
                            out=x_sl[:, col:col + 128],
                            in0=x_sl[:, col:col + 128], in1=rt[:])
                dst = cc_in[t] if t < T - 1 else Y
                nc.sync.dma_start(out=dst[:], in_=x_sl[:])
                if t < T - 1:
                    nc.gpsimd.collective_compute(
                        "AllGather",
                        mybir.AluOpType.bypass,
                        replica_groups=[list(range(NCORES))],
                        ins=[cc_in[t][:]],
                        outs=[cc_out[t][:]],
                    )
    # spread indirect gathers across the 4 SWDGE queues
    qi = 0
    for fn in nc.m.functions:
        for bb in fn.blocks:
            for ins in bb.instructions:
                if (type(ins).__name__ == "InstDMACopy"
                        and getattr(ins, "queue", "") == "qPoolDynamic"
                        and ins.engine == mybir.EngineType.Pool):
                    ins.queue = f"qPoolDynamic{qi or ''}"
                    qi = (qi + 1) % 4
    if split:
        _split_multiwaits(nc)
    return nc


class _PjrtRunner:
    """Jitted PJRT runner (mimics bass2jax.run_bass_via_pjrt) kept alive so
    repeated executions reuse the compiled NEFF."""

    def __init__(self, nc, n_cores):
        import jax
        from jax.sharding import Mesh, PartitionSpec
        from jax.experimental.shard_map import shard_map
        from concourse.bass2jax import (
            _bass_exec_p, install_neuronx_cc_hook, partition_id_tensor)

        install_neuronx_cc_hook()
        self.jax = jax
        self.n_cores = n_cores
        pname = nc.partition_id_tensor.name if nc.partition_id_tensor else None

        in_names, out_names, out_avals, zero_outs = [], [], [], []
        for alloc in nc.m.functions[0].allocations:
            if not isinstance(alloc, mybir.MemoryLocationSet):
                continue
            name = alloc.memorylocations[0].name
            if alloc.kind == "ExternalInput":
                if name != pname:
                    in_names.append(name)
            elif alloc.kind == "ExternalOutput":
                out_names.append(name)
                shape = tuple(alloc.tensor_shape)
                dtype = mybir.dt.np(alloc.dtype)
                out_avals.append(jax.core.ShapedArray(shape, dtype))
                zero_outs.append(np.zeros(shape, dtype))
        self.in_names = list(in_names)
        self.out_names = out_names
        self.out_avals = out_avals
        self.zero_outs = zero_outs
        n_params = len(in_names)
        all_names = in_names + out_names
        if pname is not None:
            all_names.append(pname)

        def _body(*args):
            operands = list(args)
            if pname is not None:
                operands.append(partition_id_tensor())
            outs = _bass_exec_p.bind(
                *operands,
                out_avals=tuple(out_avals),
                in_names=tuple(all_names),
                out_names=tuple(out_names),
                lowering_input_output_aliases=(),
                sim_require_finite=True,
                sim_require_nnan=True,
                nc=nc,
            )
            return tuple(outs)

        devices = jax.devices()[:n_cores]
        self.mesh = Mesh(np.asarray(devices), ("core",))
        in_specs = (PartitionSpec("core"),) * (n_params + len(out_names))
        out_specs = (PartitionSpec("core"),) * len(out_names)
        self.sharded = jax.jit(
            shard_map(_body, mesh=self.mesh, in_specs=in_specs,
                      out_specs=out_specs, check_rep=False),
            keep_unused=True,
        )

    def upload(self, in_maps):
        from jax.sharding import NamedSharding, PartitionSpec
        sh = NamedSharding(self.mesh, PartitionSpec("core"))
        args = []
        for name in self.in_names:
            cat = np.concatenate([np.asarray(m[name]) for m in in_maps], axis=0)
            args.append(self.jax.device_put(cat, sh))
        for z in self.zero_outs:
            cat = np.zeros((self.n_cores * z.shape[0], *z.shape[1:]), z.dtype)
            args.append(self.jax.device_put(cat, sh))
        return args

    def run(self, args):
        outs = self.sharded(*args)
        self.jax.block_until_ready(outs)
        return outs

    def timed_run(self, args, iters=3):
        outs = self.run(args)
        times = []
        for _ in range(iters):
            t0 = time.perf_counter()
            outs = self.run(args)
            times.append(time.perf_counter() - t0)
        return outs, min(times)

    def results(self, outs):
        res = []
        for c in range(self.n_cores):
            d = {}
            for i, name in enumerate(self.out_names):
                full = np.asarray(outs[i])
                per = full.reshape(self.n_cores, *self.out_avals[i].shape)
                d[name] = per[c]
            res.append(d)
        return res


_LAST_RUNNER = None


def kernel(x, edge_index, edge_attr, W, b, alpha):
    global _LAST_RUNNER
    meta = _preprocess(x, edge_index, edge_attr, W, b, alpha)
    hops = meta["hops"]

    nc = _build_kernel(meta)

    iota = np.tile(np.arange(128, dtype=np.float32)[None, :], (128, 1))
    Wflat = meta["Ws"].reshape(T * K, D, D)
    biasd = np.tile(meta["bias_rows"][:, None, :], (1, 128, 1)).astype(np.float32)

    in_maps = []
    for c in range(NCORES):
        in_maps.append({
            "X0": meta["x_pm"],
            "x0s": meta["x_pm"][c * 128:(c + 1) * 128],
            "idx1": hops[0]["idx"][c],
            "idx2": hops[1]["idx"][c],
            "rel1": hops[0]["rel"][c],
            "rel2": hops[1]["rel"][c],
            "Wd": Wflat,
            "IOTA": iota,
            "BIASD": biasd,
        })

    runner = _PjrtRunner(nc, NCORES)
    args = runner.upload(in_maps)
    outs = runner.run(args)
    results = runner.results(outs)
    _LAST_RUNNER = (runner, args)

    # Y is partition-major [128, (g d)] per core -> unpack to [NPAD, D]
    out_pm = np.stack([results[c]["Y"] for c in range(NCORES)], axis=0)
    out_pad = (out_pm.reshape(NCORES, 128, NPC // 128, D)
               .transpose(0, 2, 1, 3).reshape(NPAD, D))
    return out_pad[meta["pad_id"]]


# revision 17
# speedup vs baseline: 1.2118x; 1.2118x over previous
"""DelayGNNStage Trainium2 kernel: 3-layer, 2-hop message-passing GNN.

Strategy (graph/data parallel over 8 NeuronCores):
  - Nodes are partitioned across cores by destination (12800 padded rows each).
  - Edges are sharded by dst core, sorted by (dst superblock, dst row).
  - Per 512-row dst superblock: gather source rows ([128,1]-offset indirect
    DMAs, one per 128-edge tile), build one-hot segment matrices on-chip
    (is_equal vs an iota constant), and aggregate via TensorE matmuls into
    PSUM as aggT[d, dst_window].
  - aggT @ (softmax(alpha)-scaled W) for both hops accumulates in PSUM,
    then relu + residual on the SBUF-resident x slice.
  - Updated slices are AllGathered between layers so the next layer's
    hop-1 (and later hop-2) gathers can read the full table.
"""

import time

import numpy as np

import concourse.bass as bass
import concourse.mybir as mybir
import concourse.mybir as mb
from concourse.tile import TileContext

# problem constants (hardcoded per contract)
N, E, D, T, K, NU = 100000, 1600000, 128, 3, 2, 1
NCORES = 8
NPC = 12800          # padded nodes per core (25 superblocks x 512)
NSB = NPC // 512     # superblocks per core
NPAD = NCORES * NPC  # 102400
SEG_TILES = 8        # tiles gathered/S-built per segment


def _split_multiwaits(nc):
    """Walrus in this container only accepts one sem-wait per instruction;
    hoist extras onto same-engine NoOps immediately before."""
    for fn in nc.m.functions:
        for bb in fn.blocks:
            newinsts = []
            for ins in bb.instructions:
                si = ins.sync_info
                try:
                    waits = list(si.on_wait) if si is not None else []
                except Exception:
                    waits = []
                if len(waits) > 1:
                    keep = waits[-1]
                    for w in waits[:-1]:
                        nop = mb.InstNoOp(
                            name=nc.get_next_instruction_name(), ins=[], outs=[])
                        nop.engine = ins.engine
                        nop.sync_info = mb.SyncInfo(on_wait=[w], on_update=[])
                        newinsts.append(nop)
                    ins.sync_info = mb.SyncInfo(
                        on_wait=[keep], on_update=list(si.on_update))
                newinsts.append(ins)
            bb.instructions = newinsts


def _build_schedule(src_by, dst_by):
    """Common (cross-core) tile schedule for one hop.

    src_by/dst_by: per core, per superblock: arrays of (src_padded,
    dst_local_in_sb) sorted by dst_local.

    Returns:
      ntiles: [NSB] list of tile counts (common across cores)
      wins:   per sb, list of window bases (len ntiles[sb])
      idx:    [NCORES][128, total_tiles] int32 gather row ids (pad -> 0)
      rel:    [NCORES][128, total_tiles] f32 dst_rel in [0,128) or -1 pad
    """
    ntiles = []
    wins = []
    per_core_cols_idx = [[] for _ in range(NCORES)]
    per_core_cols_rel = [[] for _ in range(NCORES)]
    for s in range(NSB):
        ptr = [0] * NCORES
        srcs = [src_by[c][s] for c in range(NCORES)]
        dsts = [dst_by[c][s] for c in range(NCORES)]
        lens = [len(x) for x in srcs]
        sb_wins = []
        while True:
            rem = [lens[c] - ptr[c] for c in range(NCORES)]
            if max(rem) == 0:
                break
            # window base: min over cores of next dst_local
            w = min(int(dsts[c][ptr[c]]) for c in range(NCORES) if rem[c] > 0)
            w = min(w, 512 - 128)
            sb_wins.append(w)
            for c in range(NCORES):
                p0 = ptr[c]
                # fill up to 128 edges with dst_local < w + 128
                hi = min(p0 + 128, lens[c])
                d = dsts[c]
                p1 = p0
                while p1 < hi and d[p1] < w + 128:
                    p1 += 1
                cnt = p1 - p0
                coli = np.zeros(128, dtype=np.int32)
                colr = np.full(128, -1.0, dtype=np.float32)
                if cnt:
                    coli[:cnt] = srcs[c][p0:p1]
                    colr[:cnt] = d[p0:p1] - w
                per_core_cols_idx[c].append(coli)
                per_core_cols_rel[c].append(colr)
                ptr[c] = p1
        ntiles.append(len(sb_wins))
        wins.append(sb_wins)
    idx = [np.stack(per_core_cols_idx[c], axis=1) for c in range(NCORES)]
    rel = [np.stack(per_core_cols_rel[c], axis=1).astype(np.float32)
           for c in range(NCORES)]
    return ntiles, wins, idx, rel


def _preprocess(x, edge_index, edge_attr, W, b, alpha):
    """Host-side sharding/scheduling. Returns per-core input maps and the
    common schedule metadata."""
    x = np.asarray(x, dtype=np.float32)
    ei = np.asarray(edge_index)
    ea = np.asarray(edge_attr)
    W = np.asarray(W, dtype=np.float32)
    b = np.asarray(b, dtype=np.float32)
    alpha = np.asarray(alpha, dtype=np.float32)

    # softmax over k, fold into W; fold bias rows
    aexp = np.exp(alpha - alpha.max(axis=1, keepdims=True))
    a = aexp / aexp.sum(axis=1, keepdims=True)          # [T, K]
    Ws = W * a[:, :, None, None]                        # [T, K, D, D]
    bias_rows = (a[:, :, None] * b).sum(axis=1)         # [T, D]
    has_bias = bool(np.abs(bias_rows).max() > 0)

    # node padding: original v -> core v//12500, padded id c*NPC + (v - c*12500)
    orig_per_core = N // NCORES  # 12500
    v = np.arange(N)
    core_of = v // orig_per_core
    pad_id = core_of * NPC + (v - core_of * orig_per_core)

    x_pad = np.zeros((NPAD, D), dtype=np.float32)
    x_pad[pad_id] = x
    # partition-major packing: X_pm[c*128+p, g*128+d] = x_pad[c*12800+g*128+p, d]
    x_pm = (x_pad.reshape(NCORES, NPC // 128, 128, D)
            .transpose(0, 2, 1, 3).reshape(NCORES * 128, NPC // 128 * D))

    src_p = pad_id[ei[0]]
    dst_p = pad_id[ei[1]]

    hops = []
    for k in (1, 2):
        sel = ea == k
        s_k, d_k = src_p[sel], dst_p[sel]
        dcore = d_k // NPC
        dloc = d_k - dcore * NPC
        src_by = [[] for _ in range(NCORES)]
        dst_by = [[] for _ in range(NCORES)]
        for c in range(NCORES):
            m = dcore == c
            sc, dc = s_k[m], dloc[m]
            sb = dc // 512
            dl = dc - sb * 512
            order = np.lexsort((dl, sb))
            sc, sb, dl = sc[order], sb[order], dl[order]
            bounds = np.searchsorted(sb, np.arange(NSB + 1))
            for s in range(NSB):
                lo, hi = bounds[s], bounds[s + 1]
                src_by[c].append(sc[lo:hi].astype(np.int32))
                dst_by[c].append(dl[lo:hi].astype(np.int32))
        ntiles, wins, idx, rel = _build_schedule(src_by, dst_by)
        # remap padded node id -> partition-major table row (q*100+g where
        # q = c*128+p, node = c*12800 + g*128 + p)
        for c in range(NCORES):
            s_ = idx[c]
            sc_ = s_ // NPC
            rem = s_ - sc_ * NPC
            g_ = rem // 128
            p_ = rem - g_ * 128
            idx[c] = ((sc_ * 128 + p_) * (NPC // 128) + g_).astype(np.int32)
        hops.append(dict(ntiles=ntiles, wins=wins, idx=idx, rel=rel))

    return dict(x_pad=x_pad, x_pm=x_pm, pad_id=pad_id, Ws=Ws,
                bias_rows=bias_rows, has_bias=has_bias, hops=hops)


def _build_kernel(meta, split=True):
    """Emit the bass kernel for the common schedule in `meta`."""
    hops = meta["hops"]
    has_bias = meta["has_bias"]
    T1 = sum(hops[0]["ntiles"])   # total tiles hop 1
    T2 = sum(hops[1]["ntiles"])

    nc = bass.Bass(num_devices=NCORES, num_swdge_queues=4)
    f32 = mybir.dt.float32

    # partition-major table layout: row q=(c*128+p), col (g*128+d) holds
    # node c*12800+g*128+p feature d; gather view row = q*100+g
    x0s = nc.dram_tensor("x0s", [128, NPC // 128 * D], f32,
                         kind="ExternalInput")
    cc_x0 = nc.dram_tensor("cc_x0", [128, NPC // 128 * D], f32)
    X0 = nc.dram_tensor("X0i", [NCORES * 128, NPC // 128 * D], f32,
                        addr_space="Shared")
    idx1 = nc.dram_tensor("idx1", [128, T1], mybir.dt.int32, kind="ExternalInput")
    idx2 = nc.dram_tensor("idx2", [128, T2], mybir.dt.int32, kind="ExternalInput")
    rel1 = nc.dram_tensor("rel1", [128, T1], f32, kind="ExternalInput")
    rel2 = nc.dram_tensor("rel2", [128, T2], f32, kind="ExternalInput")
    Wd = nc.dram_tensor("Wd", [T * K, D, D], f32, kind="ExternalInput")
    IOTA = nc.dram_tensor("IOTA", [128, 128], f32, kind="ExternalInput")
    BIASD = nc.dram_tensor("BIASD", [T, 128, D], f32, kind="ExternalInput")
    Y = nc.dram_tensor("Y", [128, NPC // 128 * D], f32, kind="ExternalOutput")

    cc_in = [nc.dram_tensor(f"cc_in{t}", [128, NPC // 128 * D], f32)
             for t in range(T - 1)]
    cc_out = [nc.dram_tensor(f"cc_out{t}", [NCORES * 128, NPC // 128 * D], f32,
              addr_space="Shared") for t in range(T - 1)]

    # gather tables per (t, k): k=1 reads xs[t], k=2 reads xs[t-1] (t=0,1 -> x0)
    tables = {(0, 1): X0, (0, 2): X0}
    if T > 1:
        tables.update({(1, 1): cc_out[0], (1, 2): X0})
    if T > 2:
        tables.update({(2, 1): cc_out[1], (2, 2): cc_out[0]})

    idx_d = {1: idx1, 2: idx2}
    rel_d = {1: rel1, 2: rel2}

    with TileContext(nc) as tc:
        with (
            tc.tile_pool(name="const", bufs=1) as cpool,
            tc.tile_pool(name="blob", bufs=1) as bpool,
            tc.tile_pool(name="xres", bufs=1) as xpool,
            tc.tile_pool(name="agg2res", bufs=1) as a2pool,
            tc.tile_pool(name="mbuf", bufs=3) as mpool,
            tc.tile_pool(name="sbuf_s", bufs=3) as spool,
            tc.tile_pool(name="agg1", bufs=2) as a1pool,
            tc.tile_pool(name="relu", bufs=3) as rpool,
            tc.tile_pool(name="psumA", bufs=2, space="PSUM") as ppoolA,
            tc.tile_pool(name="psumB", bufs=2, space="PSUM") as ppoolB,
            tc.tile_pool(name="psumO", bufs=2, space="PSUM") as ppoolO,
        ):
            iota_sb = cpool.tile([128, 128], f32, name="iota_sb")
            nc.sync.dma_start(out=iota_sb[:], in_=IOTA[:])
            zero_sb = cpool.tile([128, 512], f32, name="zero_sb")
            nc.vector.memset(zero_sb[:], 0.0)
            w_sb = cpool.tile([128, T * K * D], f32, name="w_sb")
            nc.sync.dma_start(
                out=w_sb[:].rearrange("p (g d) -> p g d", d=D),
                in_=Wd[:].rearrange("g p d -> p g d"))
            if has_bias:
                bias_sb = cpool.tile([128, T * D], f32, name="bias_sb")
                nc.sync.dma_start(
                    out=bias_sb[:].rearrange("p (t d) -> p t d", d=D),
                    in_=BIASD[:].rearrange("t p d -> p t d"))

            idx_sb = {}
            rel_sb = {}
            for k, Tk in ((1, T1), (2, T2)):
                it = bpool.tile([128, Tk], mybir.dt.int32, name=f"idx_sb{k}")
                nc.sync.dma_start(out=it[:], in_=idx_d[k][:])
                rt = bpool.tile([128, Tk], f32, name=f"rel_sb{k}")
                nc.sync.dma_start(out=rt[:], in_=rel_d[k][:])
                idx_sb[k] = it
                rel_sb[k] = rt

            x_sl = xpool.tile([128, NPC], f32, name="x_sl")  # [p, blk*128+d]
            nc.sync.dma_start(out=x_sl[:], in_=x0s[:])
            # replicate the full x0 table to every core's HBM (collective is
            # far cheaper than uploading 8 host copies)
            nc.sync.dma_start(out=cc_x0[:], in_=x_sl[:])
            nc.gpsimd.collective_compute(
                "AllGather", mybir.AluOpType.bypass,
                replica_groups=[list(range(NCORES))],
                ins=[cc_x0[:]], outs=[X0[:]])

            agg2 = a2pool.tile([128, NSB * 512], f32, name="agg2")

            # tile column offsets per (k, sb)
            tile_off = {}
            for ki, k in enumerate((1, 2)):
                off = 0
                for s in range(NSB):
                    tile_off[(k, s)] = off
                    off += hops[ki]["ntiles"][s]

            def hop_aggregate(t, k, s, ppool, ptag):
                """Gather + segment matmuls for (layer t, hop k, superblock s).
                Returns the PSUM aggT tile [128, 512]."""
                ki = k - 1
                nt = hops[ki]["ntiles"][s]
                wins = hops[ki]["wins"][s]
                base = tile_off[(k, s)]
                psum = ppool.tile([128, 512], f32, space="PSUM",
                                  name=f"ps{t}_{k}_{s}", tag=ptag)
                nc.scalar.copy(out=psum[:], in_=zero_sb[:])
                table = tables[(t, k)][:].rearrange("q (g d) -> (q g) d", d=D)
                for g0 in range(0, nt, SEG_TILES):
                    g1 = min(g0 + SEG_TILES, nt)
                    ntg = g1 - g0
                    m = mpool.tile([128, SEG_TILES * 128], f32,
                                   name=f"m{t}_{k}_{s}_{g0}", tag="m")
                    for j in range(g0, g1):
                        nc.gpsimd.indirect_dma_start(
                            out=m[:, (j - g0) * 128:(j - g0 + 1) * 128],
                            out_offset=None,
                            in_=table,
                            in_offset=bass.IndirectOffsetOnAxis(
                                ap=idx_sb[k][:, base + j:base + j + 1], axis=0),
                        )
                    sm = spool.tile([128, SEG_TILES * 128], f32,
                                    name=f"s{t}_{k}_{s}_{g0}", tag="s")
                    nc.vector.tensor_tensor(
                        out=sm[:, :ntg * 128].rearrange("p (j c) -> p j c", c=128),
                        in0=rel_sb[k][:, base + g0:base + g1]
                            .to_broadcast([128, ntg, 128]),
                        in1=iota_sb[:].rearrange("p (j c) -> p j c", j=1)
                            .to_broadcast([128, ntg, 128]),
                        op=mybir.AluOpType.is_equal,
                    )
                    for j in range(g0, g1):
                        w = wins[j]
                        jj = j - g0
                        nc.tensor.matmul(
                            out=psum[:, w:w + 128],
                            lhsT=m[:, jj * 128:(jj + 1) * 128],
                            rhs=sm[:, jj * 128:(jj + 1) * 128],
                            start=False, stop=(j == nt - 1),
                            skip_group_check=True,
                        )
                return psum

            for t in range(T):
                # phase A: hop 2 (older table) -> resident agg2
                for s in range(NSB):
                    psA = hop_aggregate(t, 2, s, ppoolA, "psA")
                    nc.scalar.copy(out=agg2[:, s * 512:(s + 1) * 512], in_=psA[:])
                # phase B: hop 1 (fresh table) + combine + residual + store
                for s in range(NSB):
                    psB = hop_aggregate(t, 1, s, ppoolB, "psB")
                    a1 = a1pool.tile([128, 512], f32, name=f"a1_{t}_{s}", tag="a1")
                    nc.scalar.copy(out=a1[:], in_=psB[:])
                    for g in range(4):
                        op = ppoolO.tile([128, 128], f32, space="PSUM",
                                         name=f"op{t}_{s}_{g}", tag="op")
                        nc.tensor.matmul(
                            out=op[:], lhsT=a1[:, g * 128:(g + 1) * 128],
                            rhs=w_sb[:, (t * K + 0) * D:(t * K + 1) * D],
                            start=True, stop=False, skip_group_check=True)
                        nc.tensor.matmul(
                            out=op[:],
                            lhsT=agg2[:, s * 512 + g * 128: s * 512 + (g + 1) * 128],
                            rhs=w_sb[:, (t * K + 1) * D:(t * K + 2) * D],
                            start=False, stop=True, skip_group_check=True)
                        rt = rpool.tile([128, 128], f32,
                                        name=f"rt{t}_{s}_{g}", tag="rt")
                        if has_bias:
                            nc.vector.tensor_tensor(
                                out=rt[:], in0=op[:],
                                in1=bias_sb[:, t * D:(t + 1) * D],
                                op=mybir.AluOpType.add)
                            nc.scalar.activation(
                                out=rt[:], in_=rt[:],
                                func=mybir.ActivationFunctionType.Relu)
                        else:
                            nc.scalar.activation(
                                out=rt[:], in_=op[:],
                                func=mybir.ActivationFunctionType.Relu)
                        col = s * 512 + g * 128
                        nc.vector.tensor_add(
                            out=x_sl[:, col:col + 128],
                            in0=x_sl[:, col:col + 128], in1=rt[:])
                dst = cc_in[t] if t < T - 1 else Y
                nc.sync.dma_start(out=dst[:], in_=x_sl[:])
                if t < T - 1:
                    nc.gpsimd.collective_compute(
                        "AllGather",
                        mybir.AluOpType.bypass,
                        replica_groups=[list(range(NCORES))],
                        ins=[cc_in[t][:]],
                        outs=[cc_out[t][:]],
                    )
    # spread indirect gathers across the 4 SWDGE queues
    qi = 0
    for fn in nc.m.functions:
        for bb in fn.blocks:
            for ins in bb.instructions:
                if (type(ins).__name__ == "InstDMACopy"
                        and getattr(ins, "queue", "") == "qPoolDynamic"
                        and ins.engine == mybir.EngineType.Pool):
                    ins.queue = f"qPoolDynamic{qi or ''}"
                    qi = (qi + 1) % 4
    if split:
        _split_multiwaits(nc)
    return nc


class _PjrtRunner:
    """Jitted PJRT runner (mimics bass2jax.run_bass_via_pjrt) kept alive so
    repeated executions reuse the compiled NEFF."""

    def __init__(self, nc, n_cores):
        import jax
        from jax.sharding import Mesh, PartitionSpec
        from jax.experimental.shard_map import shard_map
        from concourse.bass2jax import (
            _bass_exec_p, install_neuronx_cc_hook, partition_id_tensor)

        install_neuronx_cc_hook()
        self.jax = jax
        self.n_cores = n_cores
        pname = nc.partition_id_tensor.name if nc.partition_id_tensor else None

        in_names, out_names, out_avals, zero_outs = [], [], [], []
        for alloc in nc.m.functions[0].allocations:
            if not isinstance(alloc, mybir.MemoryLocationSet):
                continue
            name = alloc.memorylocations[0].name
            if alloc.kind == "ExternalInput":
                if name != pname:
                    in_names.append(name)
            elif alloc.kind == "ExternalOutput":
                out_names.append(name)
                shape = tuple(alloc.tensor_shape)
                dtype = mybir.dt.np(alloc.dtype)
                out_avals.append(jax.core.ShapedArray(shape, dtype))
                zero_outs.append(np.zeros(shape, dtype))
        self.in_names = list(in_names)
        self.out_names = out_names
        self.out_avals = out_avals
        self.zero_outs = zero_outs
        n_params = len(in_names)
        all_names = in_names + out_names
        if pname is not None:
            all_names.append(pname)

        def _body(*args):
            operands = list(args)
            if pname is not None:
                operands.append(partition_id_tensor())
            outs = _bass_exec_p.bind(
                *operands,
                out_avals=tuple(out_avals),
                in_names=tuple(all_names),
                out_names=tuple(out_names),
                lowering_input_output_aliases=(),
                sim_require_finite=True,
                sim_require_nnan=True,
                nc=nc,
            )
            return tuple(outs)

        devices = jax.devices()[:n_cores]
        self.mesh = Mesh(np.asarray(devices), ("core",))
        in_specs = (PartitionSpec("core"),) * (n_params + len(out_names))
        out_specs = (PartitionSpec("core"),) * len(out_names)
        self.sharded = jax.jit(
            shard_map(_body, mesh=self.mesh, in_specs=in_specs,
                      out_specs=out_specs, check_rep=False),
            keep_unused=True,
        )

    def upload(self, in_maps):
        from jax.sharding import NamedSharding, PartitionSpec
        sh = NamedSharding(self.mesh, PartitionSpec("core"))
        args = []
        for name in self.in_names:
            cat = np.concatenate([np.asarray(m[name]) for m in in_maps], axis=0)
            args.append(self.jax.device_put(cat, sh))
        for z in self.zero_outs:
            cat = np.zeros((self.n_cores * z.shape[0], *z.shape[1:]), z.dtype)
            args.append(self.jax.device_put(cat, sh))
        return args

    def run(self, args):
        outs = self.sharded(*args)
        self.jax.block_until_ready(outs)
        return outs

    def timed_run(self, args, iters=3):
        outs = self.run(args)
        times = []
        for _ in range(iters):
            t0 = time.perf_counter()
            outs = self.run(args)
            times.append(time.perf_counter() - t0)
        return outs, min(times)

    def results(self, outs):
        res = []
        for c in range(self.n_cores):
            d = {}
            for i, name in enumerate(self.out_names):
                full = np.asarray(outs[i])
                per = full.reshape(self.n_cores, *self.out_avals[i].shape)
                d[name] = per[c]
            res.append(d)
        return res


_LAST_RUNNER = None


def kernel(x, edge_index, edge_attr, W, b, alpha):
    global _LAST_RUNNER
    meta = _preprocess(x, edge_index, edge_attr, W, b, alpha)
    hops = meta["hops"]

    nc = _build_kernel(meta)

    iota = np.tile(np.arange(128, dtype=np.float32)[None, :], (128, 1))
    Wflat = meta["Ws"].reshape(T * K, D, D)
    biasd = np.tile(meta["bias_rows"][:, None, :], (1, 128, 1)).astype(np.float32)

    in_maps = []
    for c in range(NCORES):
        in_maps.append({
            "x0s": meta["x_pm"][c * 128:(c + 1) * 128],
            "idx1": hops[0]["idx"][c],
            "idx2": hops[1]["idx"][c],
            "rel1": hops[0]["rel"][c],
            "rel2": hops[1]["rel"][c],
            "Wd": Wflat,
            "IOTA": iota,
            "BIASD": biasd,
        })

    runner = _PjrtRunner(nc, NCORES)
    args = runner.upload(in_maps)
    outs = runner.run(args)
    results = runner.results(outs)
    _LAST_RUNNER = (runner, args)

    # Y is partition-major [128, (g d)] per core -> unpack to [NPAD, D]
    out_pm = np.stack([results[c]["Y"] for c in range(NCORES)], axis=0)
    out_pad = (out_pm.reshape(NCORES, 128, NPC // 128, D)
               .transpose(0, 2, 1, 3).reshape(NPAD, D))
    return out_pad[meta["pad_id"]]
